# revision 2
# baseline (speedup 1.0000x reference)
"""Trainium2 Bass kernel for nn_EnsembleAdaptor: batched per-member MLP.

Per ensemble member (32 total): y = relu(x @ w1.T + b1) @ w2.T + b2
with x (512, 1024), w1 (4096, 1024), b1 (4096), w2 (1024, 4096), b2 (1024).

Sharding: pure data parallel over members - 4 members per core across 8 cores.

Precision strategy: compensated fp8 (e4m3) with DoubleRow matmuls.
The PE runs e4m3 matmuls at 2 rows/cycle (DoubleRow perf mode, contracting
two 128-deep k-planes per instruction) vs 1 row/cycle for fp16.  Plain fp8
quantization of both operands gives ~5% error (fails the 2e-2 gate), so each
operand is split into hi + lo e4m3 parts (hi = Q(a), lo = Q(a - hi)) and each
matmul is computed as

    a @ b ~= a_hi @ b_hi + a_lo @ b_hi + a_hi @ b_lo     (lo@lo dropped)

which costs 3 DoubleRow matmuls per 2 fp16 matmuls = 0.75x the PE cycles of
the fp16 kernel, with ~2e-3 end-to-end error (second order in the e4m3
quantization noise).

All scales are powers of two; the products of hi/lo parts share one scale so
all three terms accumulate into a single fp32 PSUM group.  ScalarE applies
relu(psum*alpha + b1*sh) producing h in f32; DVE then derives h_hi = e4m3(h)
and h_lo = e4m3(h - h_hi) for the layer-2 moving operand.
"""

import contextlib
import ctypes
import os
import sys
import types

import numpy as np
import ml_dtypes

import concourse.bass as bass
import concourse.tile as tile
from concourse import bacc, mybir
from concourse.bass_utils import run_bass_kernel_spmd


def _install_ntff_shim():
    """Provide antenv.axon_hooks + the ctypes NTFF profile hook when the
    image's antenv lacks them, so trace=True works under axon. Safe no-op
    on failure."""
    try:
        import antenv.axon_hooks  # noqa: F401
        return
    except ImportError:
        pass
    try:
        mod = types.ModuleType("antenv.axon_hooks")
        _state = {"hook": None}
        mod.set_axon_ntff_profile_hook = lambda h: _state.__setitem__("hook", h)
        mod.get_axon_ntff_profile_hook = lambda: _state["hook"]
        sys.modules["antenv.axon_hooks"] = mod
        import antenv
        antenv.axon_hooks = mod

        so_path = "/opt/axon/libaxon_pjrt.so"
        if not os.path.exists(so_path):
            return
        lib = ctypes.CDLL(so_path)
        if not hasattr(lib, "axon_start_nrt_profile"):
            return
        lib.axon_start_nrt_profile.argtypes = [
            ctypes.POINTER(ctypes.c_int64),
            ctypes.c_size_t,
        ]
        lib.axon_start_nrt_profile.restype = ctypes.c_int64
        lib.axon_stop_nrt_profile.argtypes = [ctypes.c_char_p]
        lib.axon_stop_nrt_profile.restype = ctypes.c_int64

        @contextlib.contextmanager
        def _hook(output_dir, device_ids):
            import jax
            jax.devices()
            if device_ids:
                ids = (ctypes.c_int64 * len(device_ids))(*device_ids)
                rc = lib.axon_start_nrt_profile(ids, len(device_ids))
            else:
                rc = lib.axon_start_nrt_profile(None, 0)
            if rc != 0:
                raise RuntimeError(f"axon_start_nrt_profile rc={rc}")
            try:
                yield
            finally:
                n = lib.axon_stop_nrt_profile(str(output_dir).encode())
                print(f"profile: {n} file(s) written to {output_dir}",
                      file=sys.stderr)

        mod.set_axon_ntff_profile_hook(_hook)
    except Exception:
        pass

B, S, DIN, H, DOUT = 32, 512, 1024, 4096, 1024
N_W1 = H * DIN
N_B1 = H
N_W2 = DOUT * H
N_B2 = DOUT

N_CORES = 8
M_PER = B // N_CORES  # members per core

DT = DIN // 128   # 8  k-planes for layer 1
JT = H // 128     # 32 j-tiles (layer-1 outputs / layer-2 k-planes)
OT = DOUT // 128  # 8  o-tiles for layer 2
SN = S            # 512 moving free dim

FP8 = mybir.dt.float8e4
F32 = mybir.dt.float32
NP_FP8 = ml_dtypes.float8_e4m3  # TRN fp8e4: max 240

# Power-of-two quantization scales (see module docstring).
SX = 16.0          # x: sigma 1 -> 16
SW1 = 1024.0       # w1: sigma .02 -> 20.5
SH = 32.0          # h: sigma ~.45 -> 14
SW2 = 1024.0       # w2
ALPHA1 = SH / (SX * SW1)   # psum1 -> scaled h
ALPHA2 = 1.0 / (SH * SW2)  # psum2 -> y

_cache = {}

DR = mybir.MatmulPerfMode.DoubleRow


def _build_nc():
    nc = bacc.Bacc("TRN2", target_bir_lowering=False, enable_partition_id=False)
    # x: planes 0..7 = hi k-planes, 8..15 = lo k-planes
    xp = nc.dram_tensor("xp", [M_PER, 128, 2 * DT, SN], FP8, kind="ExternalInput")
    # w1 per j-tile: planes 0..7 hi, 8..15 lo (contraction on partitions)
    w1p = nc.dram_tensor("w1p", [M_PER, JT, 128, 2 * DT, 128], FP8,
                         kind="ExternalInput")
    # w2 per o-tile: planes 0..31 hi, 32..63 lo
    w2p = nc.dram_tensor("w2p", [M_PER, OT, 128, 2 * JT, 128], FP8,
                         kind="ExternalInput")
    b1p = nc.dram_tensor("b1p", [M_PER, 128, JT], F32, kind="ExternalInput")
    b2p = nc.dram_tensor("b2p", [M_PER, 128, OT], F32, kind="ExternalInput")
    ytp = nc.dram_tensor("ytp", [M_PER, OT, 128, SN], F32, kind="ExternalOutput")

    relu = mybir.ActivationFunctionType.Relu
    ident = mybir.ActivationFunctionType.Identity

    def l1_matmuls(ps, w1_t, x_t, quarter=None):
        """Emit the 12 DoubleRow matmuls of one layer-1 j-tile."""
        if quarter is None:
            lo, hi = 0, SN
        else:
            lo = quarter * (SN // 4)
            hi = lo + SN // 4
        first = True
        for p in range(DT // 2):
            s0, s1 = 2 * p, 2 * p + 2
            l0, l1 = DT + 2 * p, DT + 2 * p + 2
            # hi @ hi
            nc.tensor.matmul(ps[:], w1_t[:, s0:s1, :], x_t[:, s0:s1, lo:hi],
                             start=first, stop=False, perf_mode=DR)
            first = False
            # lo(x) @ hi(w)
            nc.tensor.matmul(ps[:], w1_t[:, s0:s1, :], x_t[:, l0:l1, lo:hi],
                             start=False, stop=False, perf_mode=DR)
            # hi(x) @ lo(w)
            nc.tensor.matmul(ps[:], w1_t[:, l0:l1, :], x_t[:, s0:s1, lo:hi],
                             start=False, stop=(p == DT // 2 - 1),
                             perf_mode=DR)

    def l2_matmuls(ps, w2_t, h_t, lo, hi):
        """Emit the 48 DoubleRow matmuls of one layer-2 o-tile."""
        first = True
        for p in range(JT // 2):
            s0, s1 = 2 * p, 2 * p + 2
            l0, l1 = JT + 2 * p, JT + 2 * p + 2
            nc.tensor.matmul(ps[:], w2_t[:, s0:s1, :], h_t[:, s0:s1, lo:hi],
                             start=first, stop=False, perf_mode=DR)
            first = False
            nc.tensor.matmul(ps[:], w2_t[:, s0:s1, :], h_t[:, l0:l1, lo:hi],
                             start=False, stop=False, perf_mode=DR)
            nc.tensor.matmul(ps[:], w2_t[:, l0:l1, :], h_t[:, s0:s1, lo:hi],
                             start=False, stop=(p == JT // 2 - 1),
                             perf_mode=DR)

    with tile.TileContext(nc) as tc:
        with (
            tc.tile_pool(name="xpool", bufs=2) as xpool,
            tc.tile_pool(name="w1pool", bufs=6) as w1pool,
            tc.tile_pool(name="w2pool", bufs=3) as w2pool,
            tc.tile_pool(name="bpool", bufs=2) as bpool,
            tc.tile_pool(name="hpool", bufs=2) as hpool,
            tc.tile_pool(name="hfpool", bufs=4) as hfpool,
            tc.tile_pool(name="ypool", bufs=4) as ypool,
            tc.tile_pool(name="ps1", bufs=4, space="PSUM") as ps1pool,
            tc.tile_pool(name="ps2", bufs=4, space="PSUM") as ps2pool,
        ):
            for m in range(M_PER):
                x_t = xpool.tile([128, 2 * DT, SN], FP8)
                w1_first = w1pool.tile([128, 2 * DT, 128], FP8, tag="w1_t")
                if m == 0:
                    # Critical head path: land the first matmul's operands
                    # before the bulk; transfers run in parallel on separate
                    # HWDGE queues.
                    nc.sync.dma_start(x_t[:, 0:2, :], xp[m, :, 0:2, :])
                    nc.sync.dma_start(w1_first[:, 0:2, :], w1p[m, 0, :, 0:2, :])
                    nc.sync.dma_start(w1_first[:, 2:, :], w1p[m, 0, :, 2:, :])
                    nc.sync.dma_start(x_t[:, 2:8, :], xp[m, :, 2:8, :])
                    nc.sync.dma_start(x_t[:, 8:, :], xp[m, :, 8:, :])
                else:
                    nc.sync.dma_start(w1_first[:], w1p[m, 0])
                    nc.sync.dma_start(x_t[:], xp[m])
                b1_t = bpool.tile([128, JT], F32, tag="b1")
                nc.gpsimd.dma_start(b1_t[:], b1p[m])
                b2_t = bpool.tile([128, OT], F32, tag="b2")
                nc.gpsimd.dma_start(b2_t[:], b2p[m])

                # h: planes 0..31 = hi j-planes, 32..63 = lo j-planes
                h_t = hpool.tile([128, 2 * JT, SN], FP8)
                for jt in range(JT):
                    if jt == 0:
                        w1_t = w1_first
                    else:
                        w1_t = w1pool.tile([128, 2 * DT, 128], FP8, tag="w1_t")
                        nc.sync.dma_start(w1_t[:], w1p[m, jt])
                    ps = ps1pool.tile([128, SN], F32)
                    l1_matmuls(ps, w1_t, x_t)
                    hf = hfpool.tile([128, SN], F32)
                    nc.scalar.activation(hf[:], ps[:], relu,
                                         bias=b1_t[:, jt:jt + 1], scale=ALPHA1)
                    nc.vector.tensor_copy(h_t[:, jt, :], hf[:])
                    nc.vector.tensor_sub(h_t[:, JT + jt, :], hf[:],
                                         h_t[:, jt, :])

                for ot in range(OT):
                    w2_t = w2pool.tile([128, 2 * JT, 128], FP8)
                    nc.sync.dma_start(w2_t[:], w2p[m, ot])
                    if m == M_PER - 1 and ot == OT - 1:
                        # Last output tile: split into four 128-wide quarters
                        # so the earlier quarters' bias-add + store DMA overlap
                        # the later quarters' matmuls instead of serializing
                        # after the very last matmul.
                        for q in range(4):
                            lo = q * (SN // 4)
                            hi = lo + SN // 4
                            ps2 = ps2pool.tile([128, SN // 4], F32, tag="ps2")
                            l2_matmuls(ps2, w2_t, h_t, lo, hi)
                            y_t = ypool.tile([128, SN // 4], F32, tag="y_t")
                            nc.scalar.activation(y_t[:], ps2[:], ident,
                                                 bias=b2_t[:, ot:ot + 1],
                                                 scale=ALPHA2)
                            nc.sync.dma_start(ytp[m, ot, :, lo:hi], y_t[:])
                        continue
                    ps2 = ps2pool.tile([128, SN], F32, tag="ps2")
                    l2_matmuls(ps2, w2_t, h_t, 0, SN)
                    y_t = ypool.tile([128, SN], F32, tag="y_t")
                    nc.scalar.activation(y_t[:], ps2[:], ident,
                                         bias=b2_t[:, ot:ot + 1], scale=ALPHA2)
                    nc.sync.dma_start(ytp[m, ot], y_t[:])
    nc.compile()
    return nc


def _hi_lo(a):
    """Split a (pre-scaled) f32 array into hi + lo e4m3 parts."""
    hi = np.clip(a, -240.0, 240.0).astype(NP_FP8)
    lo = (a - hi.astype(np.float32)).astype(NP_FP8)
    return hi, lo


def _pack_core(x_flat, ensemble_weights, members):
    """Pack one core's members into the DMA-friendly device layouts."""
    n = len(members)
    xp = np.empty((n, 128, 2 * DT, SN), dtype=NP_FP8)
    w1p = np.empty((n, JT, 128, 2 * DT, 128), dtype=NP_FP8)
    w2p = np.empty((n, OT, 128, 2 * JT, 128), dtype=NP_FP8)
    b1p = np.empty((n, 128, JT), dtype=np.float32)
    b2p = np.empty((n, 128, OT), dtype=np.float32)
    for i, mem in enumerate(members):
        x = x_flat[mem].reshape(S, DIN)
        o = 0
        w1 = ensemble_weights[mem, o:o + N_W1].reshape(H, DIN); o += N_W1
        b1 = ensemble_weights[mem, o:o + N_B1]; o += N_B1
        w2 = ensemble_weights[mem, o:o + N_W2].reshape(DOUT, H); o += N_W2
        b2 = ensemble_weights[mem, o:o + N_B2]
        # xp[p, t, s] = x[s, t*128+p] * SX  (t<8 hi, t>=8 lo)
        xs = (x.reshape(S, DT, 128) * SX).transpose(2, 1, 0)  # [p, t, s]
        hi, lo = _hi_lo(np.ascontiguousarray(xs, dtype=np.float32))
        xp[i, :, :DT] = hi
        xp[i, :, DT:] = lo
        # w1p[jt, p, t, jj] = w1[jt*128+jj, t*128+p] * SW1
        ws = (w1.reshape(JT, 128, DT, 128) * SW1).transpose(0, 3, 2, 1)
        hi, lo = _hi_lo(np.ascontiguousarray(ws, dtype=np.float32))
        w1p[i, :, :, :DT] = hi
        w1p[i, :, :, DT:] = lo
        # w2p[ot, p, t, oo] = w2[ot*128+oo, t*128+p] * SW2
        ws = (w2.reshape(OT, 128, JT, 128) * SW2).transpose(0, 3, 2, 1)
        hi, lo = _hi_lo(np.ascontiguousarray(ws, dtype=np.float32))
        w2p[i, :, :, :JT] = hi
        w2p[i, :, :, JT:] = lo
        b1p[i] = b1.reshape(JT, 128).T.astype(np.float32) * SH
        b2p[i] = b2.reshape(OT, 128).T.astype(np.float32)
    return {"xp": xp, "w1p": w1p, "w2p": w2p, "b1p": b1p, "b2p": b2p}


def kernel(x_flat: np.ndarray, ensemble_weights: np.ndarray) -> np.ndarray:
    x_flat = np.asarray(x_flat, dtype=np.float32)
    ensemble_weights = np.asarray(ensemble_weights, dtype=np.float32)

    if "nc" not in _cache:
        _cache["nc"] = _build_nc()
    nc = _cache["nc"]

    in_maps = [
        _pack_core(x_flat, ensemble_weights,
                   list(range(c * M_PER, (c + 1) * M_PER)))
        for c in range(N_CORES)
    ]

    trace = bool(int(os.environ.get("KERNEL_TRACE", "0")))
    if trace:
        _install_ntff_shim()
    res = run_bass_kernel_spmd(nc, in_maps, core_ids=list(range(N_CORES)),
                               trace=trace)
    if trace:
        _cache["exec_time_ns"] = res.exec_time_ns

    out = np.empty((B, S * DOUT), dtype=np.float32)
    for c in range(N_CORES):
        ytp = res.results[c]["ytp"]  # (M_PER, OT, 128, SN)
        for i in range(M_PER):
            mem = c * M_PER + i
            # y[s, ot*128+p] = ytp[i, ot, p, s]
            out[mem] = (
                ytp[i].transpose(2, 0, 1).reshape(S * DOUT).astype(np.float32)
            )
    return out


# revision 4
# speedup vs baseline: 1.5239x; 1.5239x over previous
"""Trainium2 Bass kernel for nn_EnsembleAdaptor: batched per-member MLP.

Per ensemble member (32 total): y = relu(x @ w1.T + b1) @ w2.T + b2
with x (512, 1024), w1 (4096, 1024), b1 (4096), w2 (1024, 4096), b2 (1024).

Sharding: pure data parallel over members - 4 members per core across 8 cores.

Precision: fp16 matmuls (fp32 PSUM) for everything except NP8 of the 32
layer-2 k-planes, which run as e4m3 DoubleRow matmuls (2 k-planes per
512-cycle matmul = 2x fp16 FLOP rate).  The fp8 partial sums accumulate in a
separate PSUM bank (their power-of-two scaling differs) and are merged with
the fp16 partial sums by a DVE tensor_add.  NP8 is chosen so the end-to-end
error stays well under the 2e-2 gate (e4m3 on both operands of a full matmul
would give ~5% error; on NP8/32 of the contraction it scales as sqrt).

Schedule: the PE issues ~64 warmup matmuls on scratch SBUF at t=0 so the
tensor-engine clock ramps to max while the head DMAs land; weight tiles are
prefetched several tiles ahead on the sync queue, x on the scalar queue,
y stores on the vector queue, biases on gpsimd, so no engine queue clogs.
"""

import contextlib
import ctypes
import os
import sys
import types

import numpy as np
import ml_dtypes

import concourse.bass as bass
import concourse.tile as tile
from concourse import bacc, mybir
from concourse.bass_utils import run_bass_kernel_spmd


def _install_ntff_shim():
    """Provide antenv.axon_hooks + the ctypes NTFF profile hook when the
    image's antenv lacks them, so trace=True works under axon. Safe no-op
    on failure."""
    try:
        import antenv.axon_hooks  # noqa: F401
        return
    except ImportError:
        pass
    try:
        mod = types.ModuleType("antenv.axon_hooks")
        _state = {"hook": None}
        mod.set_axon_ntff_profile_hook = lambda h: _state.__setitem__("hook", h)
        mod.get_axon_ntff_profile_hook = lambda: _state["hook"]
        sys.modules["antenv.axon_hooks"] = mod
        import antenv
        antenv.axon_hooks = mod

        so_path = "/opt/axon/libaxon_pjrt.so"
        if not os.path.exists(so_path):
            return
        lib = ctypes.CDLL(so_path)
        if not hasattr(lib, "axon_start_nrt_profile"):
            return
        lib.axon_start_nrt_profile.argtypes = [
            ctypes.POINTER(ctypes.c_int64),
            ctypes.c_size_t,
        ]
        lib.axon_start_nrt_profile.restype = ctypes.c_int64
        lib.axon_stop_nrt_profile.argtypes = [ctypes.c_char_p]
        lib.axon_stop_nrt_profile.restype = ctypes.c_int64

        @contextlib.contextmanager
        def _hook(output_dir, device_ids):
            import jax
            jax.devices()
            if device_ids:
                ids = (ctypes.c_int64 * len(device_ids))(*device_ids)
                rc = lib.axon_start_nrt_profile(ids, len(device_ids))
            else:
                rc = lib.axon_start_nrt_profile(None, 0)
            if rc != 0:
                raise RuntimeError(f"axon_start_nrt_profile rc={rc}")
            try:
                yield
            finally:
                n = lib.axon_stop_nrt_profile(str(output_dir).encode())
                print(f"profile: {n} file(s) written to {output_dir}",
                      file=sys.stderr)

        mod.set_axon_ntff_profile_hook(_hook)
    except Exception:
        pass

B, S, DIN, H, DOUT = 32, 512, 1024, 4096, 1024
N_W1 = H * DIN
N_B1 = H
N_W2 = DOUT * H
N_B2 = DOUT

N_CORES = 8
M_PER = B // N_CORES  # members per core

DT = DIN // 128   # 8  k-planes for layer 1
JT = H // 128     # 32 j-tiles (layer-1 outputs / layer-2 k-planes)
OT = DOUT // 128  # 8  o-tiles for layer 2
SN = S            # 512 moving free dim

NP8 = 4           # layer-2 k-planes computed in e4m3 DoubleRow (must be even)
KF16 = JT - NP8   # layer-2 k-planes kept fp16

F16 = mybir.dt.float16
FP8 = mybir.dt.float8e4
F32 = mybir.dt.float32
NP_F16 = np.float16
NP_FP8 = ml_dtypes.float8_e4m3  # TRN fp8e4: max 240

SH = 32.0         # scale on the fp8 copy of h   (sigma ~.45 -> 14)
SW2 = 1024.0      # scale on the fp8 w2 planes   (sigma .02 -> 20)
ALPHA2 = 1.0 / (SH * SW2)

N_WARM = 64       # PE warmup matmuls on scratch SBUF (clock ramp + head DMA)
WARM_ROWS = 48

_cache = {}

DR = mybir.MatmulPerfMode.DoubleRow


def _build_nc():
    nc = bacc.Bacc("TRN2", target_bir_lowering=False, enable_partition_id=False)
    xp = nc.dram_tensor("xp", [M_PER, 128, DT, SN], F16, kind="ExternalInput")
    w1p = nc.dram_tensor("w1p", [M_PER, JT, 128, DT, 128], F16,
                         kind="ExternalInput")
    w2p16 = nc.dram_tensor("w2p16", [M_PER, OT, 128, KF16, 128], F16,
                           kind="ExternalInput")
    w2p8 = nc.dram_tensor("w2p8", [M_PER, OT, 128, NP8, 128], FP8,
                          kind="ExternalInput")
    b1p16 = nc.dram_tensor("b1p16", [M_PER, 128, KF16], F32,
                           kind="ExternalInput")
    b1p8 = nc.dram_tensor("b1p8", [M_PER, 128, NP8], F32, kind="ExternalInput")
    b2p = nc.dram_tensor("b2p", [M_PER, 128, OT], F32, kind="ExternalInput")
    ytp = nc.dram_tensor("ytp", [M_PER, OT, 128, SN], F32, kind="ExternalOutput")

    relu = mybir.ActivationFunctionType.Relu
    ident = mybir.ActivationFunctionType.Identity

    with tile.TileContext(nc) as tc:
        with (
            tc.tile_pool(name="xpool", bufs=2) as xpool,
            tc.tile_pool(name="w1pool", bufs=6) as w1pool,
            tc.tile_pool(name="w2pool16", bufs=4) as w2pool16,
            tc.tile_pool(name="w2pool8", bufs=4) as w2pool8,
            tc.tile_pool(name="bpool", bufs=2) as bpool,
            tc.tile_pool(name="h16pool", bufs=2) as h16pool,
            tc.tile_pool(name="h8pool", bufs=2) as h8pool,
            tc.tile_pool(name="upool", bufs=4) as upool,
            tc.tile_pool(name="ypool", bufs=4) as ypool,
            tc.tile_pool(name="scratch", bufs=1) as scratch,
            tc.tile_pool(name="ps1", bufs=4, space="PSUM") as ps1pool,
            tc.tile_pool(name="ps2", bufs=2, space="PSUM") as ps2pool,
            tc.tile_pool(name="ps8", bufs=1, space="PSUM") as ps8pool,
            tc.tile_pool(name="pswarm", bufs=1, space="PSUM") as pswarmpool,
        ):
            # ---- PE warmup: ramp the tensor-engine clock while head DMAs
            # land.  Scratch SBUF is memset (cheap, on DVE) so the matmuls
            # read defined data; the PSUM result is never read.
            sw = scratch.tile([128, 128], F16)
            sx = scratch.tile([128, WARM_ROWS], F16)
            nc.vector.memset(sw[:], 0)
            nc.vector.memset(sx[:], 0)
            psw = pswarmpool.tile([128, WARM_ROWS], F32)
            for _ in range(N_WARM):
                nc.tensor.matmul(psw[:], sw[:], sx[:], start=True, stop=True)

            # ---- DMA issue helpers (tiles keyed for later consumption) ----
            t_x, t_w1, t_w2, t_b = {}, {}, {}, {}

            def issue_x(m):
                x_t = xpool.tile([128, DT, SN], F16)
                if m == 0:
                    # split the head x across chunks so the first k-planes
                    # land before the bulk
                    nc.scalar.dma_start(x_t[:, 0:1, :], xp[m, :, 0:1, :])
                    nc.scalar.dma_start(x_t[:, 1:3, :], xp[m, :, 1:3, :])
                    nc.scalar.dma_start(x_t[:, 3:6, :], xp[m, :, 3:6, :])
                    nc.scalar.dma_start(x_t[:, 6:, :], xp[m, :, 6:, :])
                else:
                    nc.scalar.dma_start(x_t[:], xp[m])
                t_x[m] = x_t

            def issue_w1(m, jt):
                w1_t = w1pool.tile([128, DT, 128], F16, tag="w1_t")
                if m == 0 and jt == 0:
                    nc.sync.dma_start(w1_t[:, 0:2, :], w1p[m, jt, :, 0:2, :])
                    nc.sync.dma_start(w1_t[:, 2:, :], w1p[m, jt, :, 2:, :])
                else:
                    nc.sync.dma_start(w1_t[:], w1p[m, jt])
                t_w1[(m, jt)] = w1_t

            def issue_w2(m, ot):
                w16_t = w2pool16.tile([128, KF16, 128], F16, tag="w2_16")
                nc.sync.dma_start(w16_t[:], w2p16[m, ot])
                w8_t = w2pool8.tile([128, NP8, 128], FP8, tag="w2_8")
                nc.sync.dma_start(w8_t[:], w2p8[m, ot])
                t_w2[(m, ot)] = (w16_t, w8_t)

            def issue_b(m):
                b1_t = bpool.tile([128, KF16], F32, tag="b1")
                nc.gpsimd.dma_start(b1_t[:], b1p16[m])
                b1s_t = bpool.tile([128, NP8], F32, tag="b1s")
                nc.gpsimd.dma_start(b1s_t[:], b1p8[m])
                b2_t = bpool.tile([128, OT], F32, tag="b2")
                nc.gpsimd.dma_start(b2_t[:], b2p[m])
                t_b[m] = (b1_t, b1s_t, b2_t)

            # ---- head loads ----
            issue_w1(0, 0)
            issue_x(0)
            issue_b(0)
            issue_w1(0, 1)
            issue_w1(0, 2)

            W1_PREF = 3   # w1 prefetch depth (j-tiles ahead)
            W2_PREF = 2   # w2 prefetch depth (o-tiles ahead)

            for m in range(M_PER):
                x_t = t_x[m]
                b1_t, b1s_t, b2_t = t_b[m]
                h16_t = h16pool.tile([128, KF16, SN], F16)
                h8_t = h8pool.tile([128, NP8, SN], FP8)

                for jt in range(JT):
                    if jt + W1_PREF < JT:
                        issue_w1(m, jt + W1_PREF)
                    if jt == 20:
                        issue_w2(m, 0)
                    if jt == 24:
                        issue_w2(m, 1)
                    if jt == 26 and m + 1 < M_PER:
                        issue_x(m + 1)
                        issue_b(m + 1)
                    w1_t = t_w1.pop((m, jt))
                    ps = ps1pool.tile([128, SN], F32)
                    for k in range(DT):
                        nc.tensor.matmul(ps[:], w1_t[:, k, :], x_t[:, k, :],
                                         start=(k == 0), stop=(k == DT - 1))
                    if jt < KF16:
                        nc.scalar.activation(h16_t[:, jt, :], ps[:], relu,
                                             bias=b1_t[:, jt:jt + 1])
                    else:
                        j8 = jt - KF16
                        nc.scalar.activation(h8_t[:, j8, :], ps[:], relu,
                                             bias=b1s_t[:, j8:j8 + 1],
                                             scale=SH)

                for ot in range(OT):
                    if ot + W2_PREF < OT:
                        issue_w2(m, ot + W2_PREF)
                    if m + 1 < M_PER:
                        if ot == 5:
                            issue_w1(m + 1, 0)
                        elif ot == 6:
                            issue_w1(m + 1, 1)
                        elif ot == 7:
                            issue_w1(m + 1, 2)
                    w16_t, w8_t = t_w2.pop((m, ot))
                    # Last o-tile of the last member: quarter the moving dim so
                    # the act/store tail overlaps the remaining matmuls.
                    quarters = (
                        [(q * (SN // 4), (q + 1) * (SN // 4)) for q in range(4)]
                        if (m == M_PER - 1 and ot == OT - 1) else [(0, SN)]
                    )
                    for lo, hi in quarters:
                        w = hi - lo
                        ps8t = ps8pool.tile([128, w], F32, tag="ps8")
                        for p in range(NP8 // 2):
                            nc.tensor.matmul(
                                ps8t[:], w8_t[:, 2 * p:2 * p + 2, :],
                                h8_t[:, 2 * p:2 * p + 2, lo:hi],
                                start=(p == 0), stop=(p == NP8 // 2 - 1),
                                perf_mode=DR)
                        u_t = upool.tile([128, w], F32, tag="u_t")
                        nc.scalar.activation(u_t[:], ps8t[:], ident,
                                             bias=b2_t[:, ot:ot + 1],
                                             scale=ALPHA2)
                        ps2t = ps2pool.tile([128, w], F32, tag="ps2")
                        for k in range(KF16):
                            nc.tensor.matmul(ps2t[:], w16_t[:, k, :],
                                             h16_t[:, k, lo:hi],
                                             start=(k == 0),
                                             stop=(k == KF16 - 1))
                        y_t = ypool.tile([128, w], F32, tag="y_t")
                        nc.vector.tensor_add(y_t[:], u_t[:], ps2t[:])
                        nc.gpsimd.dma_start(ytp[m, ot, :, lo:hi], y_t[:])
    nc.compile()
    return nc


def _pack_core(x_flat, ensemble_weights, members):
    """Pack one core's members into the DMA-friendly device layouts."""
    n = len(members)
    xp = np.empty((n, 128, DT, SN), dtype=NP_F16)
    w1p = np.empty((n, JT, 128, DT, 128), dtype=NP_F16)
    w2p16 = np.empty((n, OT, 128, KF16, 128), dtype=NP_F16)
    w2p8 = np.empty((n, OT, 128, NP8, 128), dtype=NP_FP8)
    b1p16 = np.empty((n, 128, KF16), dtype=np.float32)
    b1p8 = np.empty((n, 128, NP8), dtype=np.float32)
    b2p = np.empty((n, 128, OT), dtype=np.float32)
    for i, mem in enumerate(members):
        x = x_flat[mem].reshape(S, DIN)
        o = 0
        w1 = ensemble_weights[mem, o:o + N_W1].reshape(H, DIN); o += N_W1
        b1 = ensemble_weights[mem, o:o + N_B1]; o += N_B1
        w2 = ensemble_weights[mem, o:o + N_W2].reshape(DOUT, H); o += N_W2
        b2 = ensemble_weights[mem, o:o + N_B2]
        # xp[p, t, s] = x[s, t*128+p]
        xp[i] = x.reshape(S, DT, 128).transpose(2, 1, 0).astype(NP_F16)
        # w1p[jt, p, t, jj] = w1[jt*128+jj, t*128+p]
        w1p[i] = (w1.reshape(JT, 128, DT, 128).transpose(0, 3, 2, 1)
                  .astype(NP_F16))
        # w2 planes: t = layer-2 contraction plane (h j-plane)
        # w2v[ot, p, t, oo] = w2[ot*128+oo, t*128+p]
        w2v = w2.reshape(OT, 128, JT, 128).transpose(0, 3, 2, 1)
        w2p16[i] = w2v[:, :, :KF16].astype(NP_F16)
        w2p8[i] = np.clip(w2v[:, :, KF16:] * SW2, -240.0, 240.0).astype(NP_FP8)
        b1t = b1.reshape(JT, 128).T.astype(np.float32)  # [128, JT]
        b1p16[i] = b1t[:, :KF16]
        b1p8[i] = b1t[:, KF16:] * SH
        b2p[i] = b2.reshape(OT, 128).T.astype(np.float32)
    return {"xp": xp, "w1p": w1p, "w2p16": w2p16, "w2p8": w2p8,
            "b1p16": b1p16, "b1p8": b1p8, "b2p": b2p}


def kernel(x_flat: np.ndarray, ensemble_weights: np.ndarray) -> np.ndarray:
    x_flat = np.asarray(x_flat, dtype=np.float32)
    ensemble_weights = np.asarray(ensemble_weights, dtype=np.float32)

    if "nc" not in _cache:
        _cache["nc"] = _build_nc()
    nc = _cache["nc"]

    in_maps = [
        _pack_core(x_flat, ensemble_weights,
                   list(range(c * M_PER, (c + 1) * M_PER)))
        for c in range(N_CORES)
    ]

    trace = bool(int(os.environ.get("KERNEL_TRACE", "0")))
    if trace:
        _install_ntff_shim()
    res = run_bass_kernel_spmd(nc, in_maps, core_ids=list(range(N_CORES)),
                               trace=trace)
    if trace:
        _cache["exec_time_ns"] = res.exec_time_ns

    out = np.empty((B, S * DOUT), dtype=np.float32)
    for c in range(N_CORES):
        ytp = res.results[c]["ytp"]  # (M_PER, OT, 128, SN)
        for i in range(M_PER):
            mem = c * M_PER + i
            # y[s, ot*128+p] = ytp[i, ot, p, s]
            out[mem] = (
                ytp[i].transpose(2, 0, 1).reshape(S * DOUT).astype(np.float32)
            )
    return out
